# revision 3
# baseline (speedup 1.0000x reference)
"""Bass/Trainium2 kernel for nn_CasualSelfAttention (B=4, T=2048, D=1024, H=16, dk=64).

v4: transport-optimized, fp16 shipping. The axon per-call wall time is
dominated by a fixed per-argument dispatch cost (~1.8ms/arg) plus ~63us/MB
of shipped bytes, so this version ships ONE fp16 input blob per core
(7.34MB) and ONE small fp16 output, with on-device AllGathers
de-duplicating the data v1 shipped redundantly:

 - blob rows 0:1536    = this core's D-half of xq^T/xk^T/xv^T of its batch
                         (fp16 — this data fits fp16's range, and fp16's
                         11-bit mantissa keeps absmax error ~4e-3 vs the
                         2e-2 gate, where bf16's 8 bits gave ~2e-2)
 - blob rows 1536:1793 = this core's quarter-shard of its head-group's
                         weights, fp16, in SBUF-tile-ready layout, plus the
                         q/k bias row stored as raw f32 bits in fp16 slots
 - on-device: 2-rank AllGather (batch pairs) reassembles X; 4-rank AllGather
   reassembles the weight set; compute matches v1 (f32r attention; fp16
   projections/WO with f32 PSUM accumulate); a 2-rank ReduceScatter (fp16)
   sums the pair's WO partials so each core outputs only its T-half:
   out [1024, 1024] fp16.

Core mapping: core c = 2*b + g (b = batch, g = head-group of 8 heads).
X AG groups [[0,1],[2,3],[4,5],[6,7]]; W AG groups [[0,2,4,6],[1,3,5,7]].
"""
import sys
import os

sys.path.insert(0, '/opt/trn_rl_repo')

import numpy as np
import orjson

import concourse.bass as bass
import concourse.tile as tile
import concourse.mybir as mybir
from concourse.bass_utils import run_bass_kernel_spmd

# ---------------------------------------------------------------- waitsplit
# The walrus build in this container accepts at most ONE semaphore wait per
# engine instruction.  Tile emits multi-wait sync_info; split the extras into
# single-wait NoOps on the same engine stream (in-order => semantically equal).
_ws_counter = [0]


_SELF_WAIT_ENGINES = ("Activation", "DVE")


def _split_instruction_waits(inst, out_list):
    si = inst.get("sync_info")
    if not si or not si.get("on_wait"):
        out_list.append(inst)
        return
    waits = si["on_wait"]
    # ACT/DVE execute strictly in order, so a compute instruction's wait on
    # its OWN engine's semaphore (slot-reuse WAW vs an older instruction on
    # the same engine) is always already satisfied — drop it instead of
    # spending a NoOp dispatch on the bottleneck ACT stream.
    eng = inst.get("engine")
    if (eng in _SELF_WAIT_ENGINES
            and inst.get("opcode") not in ("Drain", "EventSemaphore", "NoOp")):
        kept = [w for w in waits
                if w.get("ant_name", "").rsplit("_", 1)[0] != eng]
        if kept != waits:
            si = dict(si)
            si["on_wait"] = kept
            inst = dict(inst)
            inst["sync_info"] = si
            waits = kept
    if len(waits) <= 1:
        out_list.append(inst)
        return
    for w in waits[:-1]:
        _ws_counter[0] += 1
        out_list.append({
            "debug": inst.get("debug", 0),
            "engine": inst.get("engine"),
            "ins": [],
            "name": f"I-wsplit-{_ws_counter[0]}",
            "opcode": "NoOp",
            "outs": [],
            "sync_info": {"on_update": [], "on_wait": [w]},
        })
    si = dict(si)
    si["on_wait"] = [waits[-1]]
    inst = dict(inst)
    inst["sync_info"] = si
    out_list.append(inst)


def fix_multiwait_json(bir_bytes):
    d = orjson.loads(bir_bytes)
    for fn in d["functions"]:
        for bb in fn["blocks"]:
            new = []
            for inst in bb["instructions"]:
                _split_instruction_waits(inst, new)
            bb["instructions"] = new
    return orjson.dumps(d)


class WaitSplitBass(bass.Bass):
    def to_json_bytes(self):
        return fix_multiwait_json(super().to_json_bytes())


# ---------------------------------------------------------------- kernel build
P = 128
B, T, D = 4, 2048, 1024
NH_LOC = 8            # heads per core
NP = NH_LOC // 2      # head pairs per core
DK = 64
DC = D // P           # 8 d_model chunks
SC = T // P           # 16 s-chunks
NTB = T // 512        # 4 t-blocks
XROWS = 1536          # blob rows of X shard (3 x [512, 2048])
WROWS = 257           # blob rows of W shard (quarter of [1028, 2048])
f32 = mybir.dt.float32
f32r = mybir.dt.float32r
bf16 = mybir.dt.bfloat16
f16 = mybir.dt.float16
AF = mybir.ActivationFunctionType
MULT = mybir.AluOpType.mult
BYPASS = mybir.AluOpType.bypass
ADD = mybir.AluOpType.add

X_GROUPS = [[0, 1], [2, 3], [4, 5], [6, 7]]
W_GROUPS = [[0, 2, 4, 6], [1, 3, 5, 7]]

_nc_cache = [None]


def build_nc():
    if _nc_cache[0] is not None:
        return _nc_cache[0]
    nc = WaitSplitBass(num_devices=8, enable_partition_id=False)
    blob = nc.dram_tensor("blob", [XROWS + WROWS, 2048], f16,
                          kind="ExternalInput")
    out = nc.dram_tensor("out", [1024, 1024], f16, kind="ExternalOutput")

    with tile.TileContext(nc) as tc:
        with tc.tile_pool(name="dram", bufs=1, space="DRAM") as dpool, \
             tc.tile_pool(name="persist", bufs=1) as persist, \
             tc.tile_pool(name="psProj", bufs=2, space="PSUM") as psProj, \
             tc.tile_pool(name="psS", bufs=2, space="PSUM") as psS, \
             tc.tile_pool(name="psPV", bufs=1, space="PSUM") as psPV:

            # ---- collectives: reassemble X (pair AG) and W (4-rank AG) ----
            w_in = dpool.tile([WROWS, 2048], f16, name="w_in")
            nc.sync.dma_start(w_in[:], blob[XROWS:XROWS + WROWS, :])
            w_all = dpool.tile([4 * WROWS, 2048], f16, name="w_all")
            nc.gpsimd.collective_compute(
                "AllGather", BYPASS, replica_groups=W_GROUPS,
                ins=[w_in[:]], outs=[w_all[:]])

            x_in = dpool.tile([XROWS, 2048], f16, name="x_in")
            for i, eng in enumerate((nc.sync, nc.scalar, nc.gpsimd)):
                eng.dma_start(x_in[i * 512:(i + 1) * 512, :],
                              blob[i * 512:(i + 1) * 512, :])
            x_all = dpool.tile([2 * XROWS, 2048], f16, name="x_all")
            nc.gpsimd.collective_compute(
                "AllGather", BYPASS, replica_groups=X_GROUPS,
                ins=[x_in[:]], outs=[x_all[:]])

            def x_src(which, c, ts_):
                # which: 0=q 1=k 2=v; D-chunk c; time slice ts_
                r0 = XROWS * (c // 4) + which * 512 + (c % 4) * 128
                return x_all[r0:r0 + 128, ts_]

            # ---- persistent tiles ----
            qT2 = [persist.tile([P, T], f32r, tag=f"qT2_{p}", name=f"qT2_{p}")
                   for p in range(NP)]
            kT2 = [persist.tile([P, T], f32r, tag=f"kT2_{p}", name=f"kT2_{p}")
                   for p in range(NP)]
            V_aug = persist.tile([P, SC, NH_LOC, 65], f32r, name="V_aug")
            nc.vector.memset(V_aug[:, :, :, 64].bitcast(f32), 1.0)
            b_raw = persist.tile([P, 16], f16, name="b_raw")
            nc.sync.dma_start(b_raw[:], w_all[1024:1025, :])
            b_all = b_raw.bitcast(f32)
            # b_all[:, 0:4] = bq per pair, b_all[:, 4:8] = bk per pair
            ones64 = persist.tile([1, 64], f32r, name="ones64")
            nc.vector.memset(ones64.bitcast(f32), 1.0)

            # ---- phase A: projections ----
            # 4 concurrent psum groups: 2 slots borrowed from the (idle)
            # scores pool + 2 from psProj.
            def alloc4(stem):
                ps = [psS.tile([P, 1024], f32, tag="scores",
                               name=f"{stem}_s{j}")[:, 0:512] for j in range(2)]
                ps += [psProj.tile([P, 512], f32, tag="proj",
                                   name=f"{stem}_p{j}") for j in range(2)]
                return ps

            ctx_00 = persist.tile([P, 512], f16, name="ctx_00")
            with tc.tile_pool(name="pearly", bufs=2) as pearly, \
                 tc.tile_pool(name="wpool", bufs=1) as wpool, \
                 tc.tile_pool(name="xpool", bufs=8) as xpool:
                wq = wpool.tile([P, DC, 512], f16, tag="wq", name="wq")
                nc.sync.dma_start(wq[:], w_all[0:256, :])
                wk = wpool.tile([P, DC, 512], f16, tag="wk", name="wk")
                nc.sync.dma_start(wk[:], w_all[256:512, :])
                wv = wpool.tile([P, DC, 512], f16, tag="wv", name="wv")
                nc.sync.dma_start(wv[:], w_all[512:768, :])

                for tb in range(NTB):
                    ts_ = slice(tb * 512, (tb + 1) * 512)
                    # q then k: 4 pair-groups, chunk-outer accumulation
                    for qk, (w_t, which, bcol, dst) in enumerate(
                            ((wq, 0, 0, qT2), (wk, 1, 4, kT2))):
                        ps4 = alloc4(f"psqk{tb}_{qk}")
                        for c in range(DC):
                            x_c = xpool.tile([P, 512], f16, tag=f"x{qk}",
                                             name=f"x{qk}_{tb}_{c}")
                            nc.sync.dma_start(x_c[:], x_src(which, c, ts_))
                            for p in range(NP):
                                nc.tensor.matmul(
                                    ps4[p][:], w_t[:, c, p * P:(p + 1) * P],
                                    x_c[:], start=(c == 0), stop=(c == DC - 1))
                        for p in range(NP):
                            nc.vector.tensor_scalar_add(
                                dst[p][:, ts_], ps4[p][:],
                                b_all[:, bcol + p:bcol + p + 1])
                    # v: 4 t-tile groups, chunk-outer
                    ps4 = alloc4(f"psv{tb}")
                    for c in range(DC):
                        xv_c = xpool.tile([P, 512], f16, tag="xv",
                                          name=f"xv_{tb}_{c}")
                        nc.sync.dma_start(xv_c[:], x_src(2, c, ts_))
                        for ti in range(4):
                            nc.tensor.matmul(
                                ps4[ti][:], xv_c[:, ti * P:(ti + 1) * P],
                                wv[:, c], start=(c == 0), stop=(c == DC - 1))
                    for ti in range(4):
                        tt = tb * 4 + ti
                        nc.vector.tensor_copy(
                            V_aug[:, tt, :, 0:64],
                            ps4[ti][:].rearrange("p (h d) -> p h d", d=64))

                    # early attention for (t-block 0, pair 0): its PV psum is
                    # idle during phase A and every dep of s-chunk quarter tb
                    # is produced by A(tb) — run it here so ACT starts ~100us
                    # earlier instead of idling through all projections.
                    if tb == 0:
                        pv_e0 = psPV.tile([65, 512], f32, tag="pv0",
                                          name="pv0_0_0")
                        pv_e1 = psPV.tile([65, 512], f32, tag="pv1",
                                          name="pv1_0_0")
                    for sc in range(4 * tb, 4 * tb + 4):
                        ss = slice(sc * P, (sc + 1) * P)
                        ps_s = psS.tile([P, 1024], f32, tag="scores",
                                        name=f"ps_s_0_0_{sc}")
                        nc.tensor.matmul(
                            ps_s[:, 0:512], qT2[0][0:64, ss],
                            kT2[0][0:64, 0:512], start=True, stop=True,
                            tile_position=(0, 0))
                        nc.tensor.matmul(
                            ps_s[:, 512:1024], qT2[0][64:128, ss],
                            kT2[0][64:128, 0:512], start=True, stop=True,
                            tile_position=(64, 0))
                        p_t = pearly.tile([P, 1024], f32r, tag="pe",
                                          name=f"pe_{sc}")
                        nc.scalar.activation(p_t[:], ps_s[:], AF.Exp,
                                             scale=0.125)
                        nc.tensor.matmul(
                            pv_e0[:], V_aug[:, sc, 0, :], p_t[:, 0:512],
                            start=(sc == 0), stop=(sc == SC - 1))
                        nc.tensor.matmul(
                            pv_e1[:], V_aug[:, sc, 1, :], p_t[:, 512:1024],
                            start=(sc == 0), stop=(sc == SC - 1))

            # ---- phases B+C per t-block ----
            op_d = dpool.tile([T, D], f16, name="op_d")
            with tc.tile_pool(name="ppool", bufs=5) as ppool, \
                 tc.tile_pool(name="rbpool", bufs=2) as rbpool, \
                 tc.tile_pool(name="ctxpool", bufs=2) as ctxpool, \
                 tc.tile_pool(name="wopool", bufs=1) as wopool, \
                 tc.tile_pool(name="opool", bufs=3) as opool:
                    wo = wopool.tile([P, NP, D], f16, name="wo")
                    nc.sync.dma_start(wo[:], w_all[768:1024, :])

                    def flush_evac(pend):
                        # normalize pair into its ctx tile:
                        # ctx[h] = pv[h][0:64] * bcast(1 / pv[h][64])
                        tb, p, pv0, pv1, ctx_p = pend
                        for h, pv in ((0, pv0), (1, pv1)):
                            r_t = rbpool.tile([1, 512], f32r, tag="r",
                                              name=f"r_{tb}_{p}_{h}")
                            with nc.allow_low_precision(reason="softmax recip"):
                                nc.vector.reciprocal(r_t[:], pv[64:65, :])
                            ps_rb = psProj.tile([64, 512], f32, tag="proj",
                                                name=f"ps_rb_{tb}_{p}_{h}")
                            nc.tensor.matmul(ps_rb[:], ones64[:], r_t[:],
                                             start=True, stop=True)
                            rb_s = rbpool.tile([64, 512], f32, tag="rb",
                                               name=f"rb_{tb}_{p}_{h}")
                            nc.vector.tensor_copy(rb_s[:], ps_rb[:])
                            nc.vector.tensor_tensor(
                                ctx_p[h * 64:(h + 1) * 64, :],
                                pv[0:64, :], rb_s[:], MULT)

                    def emit_wo_chunk(wtb, wctx, ti, ob):
                        # one [128t, 512o] WO output tile of t-block wtb
                        ps_o = psProj.tile([P, 512], f32, tag="proj",
                                           name=f"ps_o_{wtb}_{ti}_{ob}")
                        for p in range(NP):
                            nc.tensor.matmul(
                                ps_o[:], wctx[p][:, ti * P:(ti + 1) * P],
                                wo[:, p, ob * 512:(ob + 1) * 512],
                                start=(p == 0), stop=(p == NP - 1))
                        o_t = opool.tile([P, 512], f16, tag="o",
                                         name=f"o_{wtb}_{ti}_{ob}")
                        nc.vector.tensor_copy(o_t[:], ps_o[:])
                        nc.sync.dma_start(
                            op_d[wtb * 512 + ti * P: wtb * 512 + (ti + 1) * P,
                                 ob * 512:(ob + 1) * 512], o_t[:])

                    # (t-block 0, pair 0) already ran during phase A; seed its
                    # deferred evacuation so pair 1's sc==2 flush handles it.
                    pending = (0, 0, pv_e0, pv_e1, ctx_00)
                    pending_wo = None     # (tb, ctx_tb) whose WO is deferred
                    for tb in range(NTB):
                        ts_ = slice(tb * 512, (tb + 1) * 512)
                        ctx_tb = [ctx_00] if tb == 0 else []
                        for p in range(NP):
                            if tb == 0 and p == 0:
                                continue
                            # -- B: attention for (pair p, t-block tb) --
                            pv0 = psPV.tile([65, 512], f32, tag="pv0",
                                            name=f"pv0_{tb}_{p}")
                            pv1 = psPV.tile([65, 512], f32, tag="pv1",
                                            name=f"pv1_{tb}_{p}")
                            # Defer the previous pair's PV-psum evacuation (and
                            # the previous t-block's WO chunks) past this
                            # pair's first score/exp groups so ACT stays fed
                            # while PE runs the evac/WO work in its slack.
                            stash = []
                            for sc in range(SC):
                                ss = slice(sc * P, (sc + 1) * P)
                                ps_s = psS.tile([P, 1024], f32, tag="scores",
                                                name=f"ps_s_{tb}_{p}_{sc}")
                                nc.tensor.matmul(
                                    ps_s[:, 0:512], qT2[p][0:64, ss],
                                    kT2[p][0:64, ts_], start=True, stop=True,
                                    tile_position=(0, 0))
                                nc.tensor.matmul(
                                    ps_s[:, 512:1024], qT2[p][64:128, ss],
                                    kT2[p][64:128, ts_], start=True, stop=True,
                                    tile_position=(64, 0))
                                p_t = ppool.tile([P, 1024], f32r, tag="p",
                                                 name=f"p_{tb}_{p}_{sc}")
                                nc.scalar.activation(p_t[:], ps_s[:], AF.Exp,
                                                     scale=0.125)
                                if pending is not None and sc < 2:
                                    stash.append((sc, p_t))
                                    continue
                                if pending is not None and sc == 2:
                                    flush_evac(pending)
                                    pending = None
                                for s0, pt0 in stash:
                                    nc.tensor.matmul(
                                        pv0[:], V_aug[:, s0, 2 * p, :],
                                        pt0[:, 0:512],
                                        start=(s0 == 0), stop=False)
                                    nc.tensor.matmul(
                                        pv1[:], V_aug[:, s0, 2 * p + 1, :],
                                        pt0[:, 512:1024],
                                        start=(s0 == 0), stop=False)
                                stash = []
                                nc.tensor.matmul(
                                    pv0[:], V_aug[:, sc, 2 * p, :],
                                    p_t[:, 0:512],
                                    start=(sc == 0), stop=(sc == SC - 1))
                                nc.tensor.matmul(
                                    pv1[:], V_aug[:, sc, 2 * p + 1, :],
                                    p_t[:, 512:1024],
                                    start=(sc == 0), stop=(sc == SC - 1))
                                # sprinkle the previous t-block's 8 WO
                                # chunks across pairs 0-1, every other sc,
                                # to stay under the ACT rate per slot
                                if (pending_wo is not None and p <= 1
                                        and 2 <= sc <= 9 and (sc % 2) == 0):
                                    widx = p * 4 + (sc - 2) // 2
                                    emit_wo_chunk(pending_wo[0], pending_wo[1],
                                                  widx // 2, widx % 2)
                                    if widx == 7:
                                        pending_wo = None
                            ctx_p = ctxpool.tile([P, 512], f16, tag=f"ctx{p}",
                                                 name=f"ctx_{tb}_{p}")
                            pending = (tb, p, pv0, pv1, ctx_p)
                            ctx_tb.append(ctx_p)
                        pending_wo = (tb, ctx_tb)

                    # tail: last pair's evac + last t-block's WO
                    if pending is not None:
                        flush_evac(pending)
                        pending = None
                    if pending_wo is not None:
                        for ti in range(4):
                            for ob in range(2):
                                emit_wo_chunk(pending_wo[0], pending_wo[1],
                                              ti, ob)
                        pending_wo = None

            # ---- pair-sum the WO partials on device; each core keeps its
            # T-half of the final output ----
            rs_d = dpool.tile([1024, 1024], f16, name="rs_d")
            nc.gpsimd.collective_compute(
                "ReduceScatter", ADD, replica_groups=X_GROUPS,
                ins=[op_d[:]], outs=[rs_d[:]])
            for i, eng in enumerate((nc.sync, nc.scalar)):
                eng.dma_start(out[i * 512:(i + 1) * 512, :],
                              rs_d[i * 512:(i + 1) * 512, :])
    _nc_cache[0] = nc
    return nc


# ---------------------------------------------------------------- host side
def make_in_maps(keys, queries, values, WK_w, WK_b, WQ_w, WQ_b, WV_w, WV_b,
                 WO_w):
    keys = np.asarray(keys, dtype=np.float32)
    queries = np.asarray(queries, dtype=np.float32)
    values = np.asarray(values, dtype=np.float32)
    xq_b = [np.ascontiguousarray(queries[b].T).astype(np.float16) for b in range(B)]
    xk_b = [np.ascontiguousarray(keys[b].T).astype(np.float16) for b in range(B)]
    xv_b = [np.ascontiguousarray(values[b].T).astype(np.float16) for b in range(B)]

    def sb_pack(w_t, nchunk, ncol):
        # [rows, ncol] -> SBUF-tile layout [128, nchunk, ncol] -> [256, 2048]
        return np.ascontiguousarray(w_t).astype(np.float16) \
                 .reshape(nchunk, P, ncol).transpose(1, 0, 2).reshape(256, 2048)

    w_packs = []
    for g in range(2):
        sl = slice(512 * g, 512 * (g + 1))
        wq_sb = sb_pack(np.asarray(WQ_w, np.float32)[sl, :].T, DC, 512)
        wk_sb = sb_pack(np.asarray(WK_w, np.float32)[sl, :].T, DC, 512)
        wv_sb = sb_pack(np.asarray(WV_w, np.float32)[sl, :].T, DC, 512)
        wo_sb = sb_pack(np.asarray(WO_w, np.float32)[:, sl].T, NP, 1024)
        bias_f32 = np.ascontiguousarray(np.concatenate(
            [np.asarray(WQ_b, np.float32)[sl].reshape(NP, P).T,
             np.asarray(WK_b, np.float32)[sl].reshape(NP, P).T], axis=1))
        bias_row = bias_f32.view(np.uint16).reshape(1, 2048).view(np.float16)
        pad = np.zeros((3, 2048), np.float16)
        w_packs.append(np.concatenate(
            [wq_sb, wk_sb, wv_sb, wo_sb, bias_row, pad], axis=0))  # [1028, 2048]

    in_maps = []
    for c in range(8):
        b, g = c // 2, c % 2
        dsl = slice(512 * g, 512 * (g + 1))
        x_part = np.concatenate(
            [xq_b[b][dsl, :], xk_b[b][dsl, :], xv_b[b][dsl, :]], axis=0)
        w_part = w_packs[g][WROWS * b: WROWS * (b + 1), :]
        blob = np.concatenate([x_part, w_part], axis=0)  # [1793, 2048] f16
        in_maps.append({"blob": blob})
    return in_maps


def kernel(keys, queries, values, pad_mask, WK_w, WK_b, WQ_w, WQ_b, WV_w, WV_b,
           WO_w, WO_b):
    nc = build_nc()
    in_maps = make_in_maps(keys, queries, values, WK_w, WK_b, WQ_w, WQ_b,
                           WV_w, WV_b, WO_w)
    res = run_bass_kernel_spmd(nc, in_maps, list(range(8)))
    # Drop this call's cached executable promptly: a second, differently-built
    # executable of the same collectives NEFF in one process desyncs the mesh,
    # so leave as little loaded state behind as possible.
    import jax
    import gc
    jax.clear_caches()
    gc.collect()
    # free-dim biases folded on host: WO_b directly; WV_b exactly via
    # sum_g (WV_b_g @ WO_g^T) = WV_b @ WO_w^T  (attention rows sum to 1).
    bias = (np.asarray(WO_b, np.float64)
            + np.asarray(WV_b, np.float64) @ np.asarray(WO_w, np.float64).T)
    out = np.empty((B, T, D), np.float32)
    for b in range(B):
        full = np.concatenate(
            [res.results[2 * b]["out"], res.results[2 * b + 1]["out"]], axis=0)
        out[b] = (full.astype(np.float64) + bias).astype(np.float32)
    return out
